# revision 16
# baseline (speedup 1.0000x reference)
"""Trainium2 Bass kernel for nn_Cross_Attention (B=8, N=2048, D=768).

Math (per batch b):
    A   = softmax(t, axis=-1) = E/R     (E = exp(t), R = rowsum)
    Q   = softmax(t, axis=0)  = E/S     (S = colsum)
    attn = (x @ A^T) @ Q = x @ KQ,   KQ[d,d'] = (sum_m E[m,d] E[m,d']/R[m]) / S[d']
    out = f*(attn_1 @ W1^T + b1) + f*(attn_2 @ W2^T + b2) + x
        = x @ Msum + fb + x
    Msum = f*(KQ_1 @ W1^T + KQ_2 @ W2^T),  fb = f*(b1 + b2),  f = sigmoid(w)

All heavy matmuls run in fp8e4 with MatmulPerfMode.DoubleRow (2 k-tiles
per instruction, 2x PE rate).  The 1/R row normalization is applied to
the *stationary* operand only (sc = E * C1/R), so the KQ Gram matmuls
for token-tile i depend only on tile i's exp + rowsum: the exp, colsum
and KQ phases stream together per 2-tile pair with no sqrt barrier.

Scale plan (fp8 ranges):
    g   = E                    in [0.008, 250]    fp8 ok
    sc  = E * C1/R, C1=1024    <= ~280            fp8 ok
    kq_ps = sum sc*g = C1 * KQraw                 psum f32
    kqt = kq_ps * (CKQ/C1)/S   = CKQ*KQ, CKQ=256  ~0.3  fp8
    wts = 32*f*W^T             ~ +-0.9            fp8 (cast on host)
    msum_ps = CKQ*32 * Msum = 8192*Msum           psum f32
    msum = msum_ps/8 = 1024*Msum                  ~ +-2.7 fp8
    x8  = 8*x                                     fp8
    y_ps = x8 @ msum + (ones/128)^T @ (8192*fb)   = 8192*(x@Msum + fb)
    out = y_ps/8192 + x   (exact f32 residual), written bf16

Distribution: pure data-parallel - batch b -> core b, no collectives.
Inputs stream 2 tiles per DMA alternating over both HWDGE queues
(sync + scalar); x prefetches during the Msum phase; outputs ride
gpsimd cast-DMAs (f32 SBUF -> bf16 HBM).
"""

import numpy as np
import ml_dtypes

import concourse.bass as bass
import concourse.tile as tile
from concourse import bacc
from concourse import mybir
from concourse.bass_utils import run_bass_kernel_spmd

F32 = mybir.dt.float32
BF16 = mybir.dt.bfloat16
FP8 = mybir.dt.float8e4
DR = mybir.MatmulPerfMode.DoubleRow

B = 8
P = 128
D = 768
DT = D // P  # 6 feature tiles
C1 = 1024.0
CKQ = 256.0
CW = 32.0
CM_DIV = 8.0          # msum = msum_ps / 8
CY = 8.0              # x8 = 8*x
Y_SCALE = 1.0 / (CKQ * CW / CM_DIV * CY)  # 1/8192

# moving-dim chunks: each must stay inside one PSUM bank (512 f32)
CHUNKS = ((0, 512), (512, 256))
# upper-triangle chunk plan per d'-tile (cover columns >= dp*128) and the
# psum offset of each chunk inside the [P, 3072] kq accumulation tile.
# psum offsets chosen so no chunk crosses a 512-f32 bank boundary.
SYM_PLAN = {
    0: (((0, 512, 0), (512, 256, 512))),
    1: (((128, 384, 1024), (512, 256, 768))),
    2: (((256, 256, 1536), (512, 256, 1792))),
    3: (((256, 256, 2048), (512, 256, 2304))),
    4: (((512, 256, 2560),)),
    5: (((512, 256, 2816),)),
}
# contiguous (dst_off, src_psum_off, width) copy-out plan per dp
KQ_COPY = {
    0: ((0, 0, 768),),
    1: ((128, 1024, 384), (512, 768, 256)),
    2: ((256, 1536, 512),),
    3: ((256, 2048, 512),),
    4: ((512, 2560, 256),),
    5: ((512, 2816, 256),),
}
SYM_FILLS = [
    (1, 0), (2, 0), (2, 1), (3, 0), (3, 1),
    (4, 0), (4, 1), (4, 2), (4, 3),
    (5, 0), (5, 1), (5, 2), (5, 3),
]


def build_nc(NT=16):
    """Build the single-core Bass program.  NT = number of 128-token tiles."""
    N = NT * P
    NP = NT // 2  # tile pairs
    nc = bacc.Bacc()

    x_d = nc.dram_tensor("x", [N, D], F32, kind="ExternalInput")
    x2_d = nc.dram_tensor("x2", [N, D], F32, kind="ExternalInput")
    x3_d = nc.dram_tensor("x3", [N, D], F32, kind="ExternalInput")
    wt1_d = nc.dram_tensor("wt1", [D, D], FP8, kind="ExternalInput")  # 32*f*W1^T
    wt2_d = nc.dram_tensor("wt2", [D, D], FP8, kind="ExternalInput")  # 32*f*W2^T
    fb_d = nc.dram_tensor("fb", [1, D], F32, kind="ExternalInput")  # 8192*f*(b1+b2)
    id_d = nc.dram_tensor("ident", [P, P], F32, kind="ExternalInput")  # np.eye
    out_d = nc.dram_tensor("out", [N, D], BF16, kind="ExternalOutput")

    # pair-granular views: [pair, p, 2, d]
    x2_pr = x2_d.rearrange("(q t p) d -> q p t d", t=2, p=P)
    x3_pr = x3_d.rearrange("(q t p) d -> q p t d", t=2, p=P)
    x_pr = x_d.rearrange("(q t p) d -> q p t d", t=2, p=P)
    att_pr = [x2_pr, x3_pr]
    out_t = out_d.rearrange("(t p) d -> t p d", p=P)

    with tile.TileContext(nc) as tc:
        with (
            # one statically packed PSUM region (all 8 banks), sliced manually
            tc.tile_pool(name="ps", bufs=1, space="PSUM") as psp,
            tc.tile_pool(name="consts", bufs=1) as consts,
            tc.tile_pool(name="big", bufs=2) as big,
            tc.tile_pool(name="stream", bufs=4) as stream,
            tc.tile_pool(name="scp", bufs=2) as scp,
            tc.tile_pool(name="stats", bufs=2) as stats,
            tc.tile_pool(name="xtip", bufs=3) as xtip,
            tc.tile_pool(name="outp", bufs=3) as outp,
        ):
            # ---- PSUM layout (f32 offsets; banks are 512 f32) ----
            # [0..3072)   KQ upper-triangle chunks (phase A) /
            #             m_ps, y_ps double-buffers at 0 and 1024 (B, C)
            # [3072..3840) s_ps colsum accumulator
            # [3840..4096) two [P,128] transpose slots
            psb = psp.tile([P, 4096], F32)
            TP_OFF = (3840, 3968)

            # ---- constants ----
            ones8 = consts.tile([P, 2, P], FP8)
            nc.vector.memset(ones8, 1.0)
            onesfb = consts.tile([P, P], BF16)
            nc.vector.memset(onesfb, 1.0 / 128.0)
            ident = consts.tile([P, P], F32)
            nc.sync.dma_start(out=ident, in_=id_d[:, :])
            ident8 = consts.tile([P, P], FP8)
            nc.vector.tensor_copy(ident8, ident)
            fbb = consts.tile([P, D], F32)
            nc.sync.dma_start(out=fbb, in_=fb_d[0:1, :].to_broadcast([P, D]))
            fbby = consts.tile([P, D], BF16)
            nc.vector.tensor_copy(fbby, fbb)
            # fp8 weights [p, t, dp, j]
            wts = consts.tile([P, 2, DT, D], FP8)
            for t, wd in enumerate((wt1_d, wt2_d)):
                nc.gpsimd.dma_start(
                    out=wts[:, t], in_=wd.rearrange("(c p) j -> p c j", p=P)
                )
            # full x (f32) for the y phase + exact residual; pairs 0-3
            # prefetch early on the gpsimd SW queue, pairs 4-7 ride the
            # HWDGE queues behind x3 (issued in the t==1 block below)
            xbig = consts.tile([P, NT, D], F32)
            for q in range(4):
                nc.gpsimd.dma_start(out=xbig[:, 2 * q : 2 * q + 2, :],
                                    in_=x_pr[q])
            # scaled KQ^T per attention: kqt[t][:, dp, d] (d' on partitions)
            kqt = [
                consts.tile([P, DT, D], FP8, tag=f"kqt{t}", name=f"kqt{t}")
                for t in range(2)
            ]
            msum = consts.tile([P, DT, D], FP8)
            # (CKQ/C1)/S column scalars, per attention and d'-tile
            rscol = consts.tile([P, 2, DT], F32)

            # ---- per-attention phases: exp + colsum + KQ streamed ----
            for t in range(2):
                g = big.tile([P, NT, D], FP8, tag="big", name=f"g{t}")
                rvec = stats.tile([P, NT], F32, tag="rvec")
                rrec = stats.tile([P, NT], F32, tag="rrec")
                s_ps = psb[:, 3072:3840]
                kq_ps = psb[:, 0:3072]
                for q in range(NP):
                    xi = stream.tile([P, 2, D], F32, tag="in")
                    eng = nc.sync if q % 2 == 0 else nc.scalar
                    eng.dma_start(out=xi, in_=att_pr[t][q])
                    sc = scp.tile([P, 2, D], FP8, tag="sc")
                    for j in range(2):
                        i = 2 * q + j
                        nc.scalar.activation(
                            out=g[:, i, :], in_=xi[:, j, :],
                            func=mybir.ActivationFunctionType.Exp,
                            accum_out=rvec[:, i : i + 1],
                        )
                    nc.vector.reciprocal(rrec[:, 2 * q : 2 * q + 2],
                                         rvec[:, 2 * q : 2 * q + 2])
                    for j in range(2):
                        i = 2 * q + j
                        nc.vector.tensor_scalar(
                            out=sc[:, j, :], in0=g[:, i, :],
                            scalar1=rrec[:, i : i + 1], scalar2=C1,
                            op0=mybir.AluOpType.mult, op1=mybir.AluOpType.mult,
                        )
                    # colsum S accumulates via ones-matmul (DoubleRow pair)
                    for off, sz in CHUNKS:
                        nc.tensor.matmul(
                            s_ps[:, off : off + sz],
                            ones8,
                            g[:, 2 * q : 2 * q + 2, off : off + sz],
                            start=(q == 0), stop=(q == NP - 1),
                            perf_mode=DR,
                        )
                    # KQ upper block-triangle, 2 token-tiles per matmul
                    for dp in range(DT):
                        lhsT = sc[:, :, dp * P : (dp + 1) * P]
                        for off, sz, poff in SYM_PLAN[dp]:
                            nc.tensor.matmul(
                                kq_ps[:, poff : poff + sz],
                                lhsT,
                                g[:, 2 * q : 2 * q + 2, off : off + sz],
                                start=(q == 0), stop=(q == NP - 1),
                                perf_mode=DR,
                            )

                # 1/S (scaled) + S*4 for the symmetric fills
                rsb = stream.tile([P, D], F32, tag="rsb", bufs=2)
                nc.vector.reciprocal(rsb, s_ps)
                nc.vector.tensor_scalar_mul(rsb, rsb, CKQ / C1)
                ssb = stream.tile([P, D], F32, tag="ssb", bufs=2)
                nc.vector.tensor_scalar_mul(ssb, s_ps, C1 / CKQ)
                for c in range(DT):
                    tp = psb[:, TP_OFF[c % 2] : TP_OFF[c % 2] + P]
                    nc.tensor.transpose(tp, rsb[:, c * P : (c + 1) * P], ident)
                    nc.vector.tensor_copy(rscol[:, t, c : c + 1], tp[:, 0:1])

                # upper blocks: scale rows by (CKQ/C1)/S[d'] on the psum copy
                for dp in range(DT):
                    for dst, src, w in KQ_COPY[dp]:
                        nc.vector.tensor_scalar_mul(
                            kqt[t][:, dp, dst : dst + w],
                            kq_ps[:, src : src + w],
                            rscol[:, t, dp : dp + 1],
                        )
                # lower blocks = transposed upper blocks rescaled:
                # kqt[hi][p, lo*P+q] = tp[p, q] * S[lo*P+q] / S[hi*P+p]
                for fi, (hi, lo) in enumerate(SYM_FILLS):
                    tp = psb[:, TP_OFF[fi % 2] : TP_OFF[fi % 2] + P]
                    # transpose via plain matmul (lhsT^T @ I) into f32 psum
                    nc.tensor.matmul(
                        tp, kqt[t][:, lo, hi * P : (hi + 1) * P], ident8,
                        start=True, stop=True,
                    )
                    nc.vector.scalar_tensor_tensor(
                        out=kqt[t][:, hi, lo * P : (lo + 1) * P],
                        in0=tp,
                        scalar=rscol[:, t, hi : hi + 1],
                        in1=ssb[:, lo * P : (lo + 1) * P],
                        op0=mybir.AluOpType.mult,
                        op1=mybir.AluOpType.mult,
                    )

                # x pairs 4-7 queue behind x3 on the HWDGE queues
                if t == 1:
                    for q in range(4, NP):
                        eng = nc.sync if q % 2 == 0 else nc.scalar
                        eng.dma_start(out=xbig[:, 2 * q : 2 * q + 2, :],
                                      in_=x_pr[q])

            # ---- Msum[d, j] = sum_t sum_d' kqt[t][d', d] * wts[t][d', j] ----
            for d in range(DT):
                # double-buffer at psum offsets 0/1024 so chain d+1 overlaps
                # chain d's copy-out
                mb = 1024 * (d % 2)
                m_ps = psb[:, mb : mb + D]
                for t in range(2):
                    for dpp in range(0, DT, 2):
                        lhsT = kqt[t][:, dpp : dpp + 2, d * P : (d + 1) * P]
                        for off, sz in CHUNKS:
                            nc.tensor.matmul(
                                m_ps[:, off : off + sz],
                                lhsT,
                                wts[:, t, dpp : dpp + 2, off : off + sz],
                                start=(t == 0 and dpp == 0),
                                stop=(t == 1 and dpp == DT - 2),
                                perf_mode=DR,
                            )
                nc.vector.tensor_scalar_mul(msum[:, d, :], m_ps, 1.0 / CM_DIV)

            # ---- y = x8 @ msum + fb_mm; out = y/8192 + x ----
            for i in range(NT):
                x8 = scp.tile([P, D], FP8, tag="x8", bufs=3)
                nc.vector.tensor_scalar_mul(x8, xbig[:, i, :], CY)
                xti = xtip.tile([P, DT, P], FP8, tag="xti")
                for c in range(DT):
                    tp = psb[:, TP_OFF[c % 2] : TP_OFF[c % 2] + P]
                    nc.tensor.matmul(
                        tp, x8[:, c * P : (c + 1) * P], ident8,
                        start=True, stop=True,
                    )
                    nc.scalar.copy(xti[:, c, :], tp)
                yb = 1024 * (i % 2)
                y_ps = psb[:, yb : yb + D]
                for off, sz in CHUNKS:
                    nc.tensor.matmul(
                        y_ps[:, off : off + sz], onesfb, fbby[:, off : off + sz],
                        start=True, stop=False,
                    )
                for k in range(0, DT, 2):
                    for off, sz in CHUNKS:
                        nc.tensor.matmul(
                            y_ps[:, off : off + sz],
                            xti[:, k : k + 2, :],
                            msum[:, k : k + 2, off : off + sz],
                            start=False, stop=(k == DT - 2),
                            perf_mode=DR,
                        )
                oi = outp.tile([P, D], F32, tag="out")
                nc.vector.scalar_tensor_tensor(
                    out=oi, in0=y_ps, scalar=Y_SCALE, in1=xbig[:, i, :],
                    op0=mybir.AluOpType.mult, op1=mybir.AluOpType.add,
                )
                # f32 -> bf16 cast rides the gpsimd store
                nc.gpsimd.dma_start(out=out_t[i], in_=oi)

    nc.compile()
    return nc


def prep_inputs(inputs):
    x = np.ascontiguousarray(np.asarray(inputs["x"], dtype=np.float32))
    x2 = np.ascontiguousarray(np.asarray(inputs["x2"], dtype=np.float32))
    x3 = np.ascontiguousarray(np.asarray(inputs["x3"], dtype=np.float32))
    W1 = np.asarray(inputs["W1"], dtype=np.float32)
    b1 = np.asarray(inputs["b1"], dtype=np.float32)
    W2 = np.asarray(inputs["W2"], dtype=np.float32)
    b2 = np.asarray(inputs["b2"], dtype=np.float32)
    w = np.asarray(inputs["w"], dtype=np.float32)

    f = 1.0 / (1.0 + np.exp(-float(w.reshape(-1)[0])))
    wt1 = np.ascontiguousarray((CW * f * W1).T).astype(ml_dtypes.float8_e4m3fn)
    wt2 = np.ascontiguousarray((CW * f * W2).T).astype(ml_dtypes.float8_e4m3fn)
    fb = (f * (b1 + b2) / Y_SCALE).astype(np.float32).reshape(1, D)

    ident = np.eye(P, dtype=np.float32)
    return [
        {
            "x": x[b], "x2": x2[b], "x3": x3[b],
            "wt1": wt1, "wt2": wt2, "fb": fb, "ident": ident,
        }
        for b in range(B)
    ]


_NC = None


def kernel(**inputs) -> np.ndarray:
    global _NC
    if _NC is None:
        _NC = build_nc()
    in_maps = prep_inputs(inputs)
    res = run_bass_kernel_spmd(_NC, in_maps, list(range(B)))
    return np.stack(
        [res.results[b]["out"].astype(np.float32) for b in range(B)], axis=0
    )


# revision 19
# speedup vs baseline: 1.4683x; 1.4683x over previous
"""Trainium2 Bass kernel for nn_Cross_Attention (B=8, N=2048, D=768).

Math (per batch b):
    A   = softmax(t, axis=-1) = E/R     (E = exp(t), R = rowsum)
    Q   = softmax(t, axis=0)  = E/S     (S = colsum)
    attn = (x @ A^T) @ Q = x @ KQ,   KQ[d,d'] = (sum_m E[m,d] E[m,d']/R[m]) / S[d']
    out = x @ Msum + fb + x
    Msum = f*(KQ_1 @ W1^T + KQ_2 @ W2^T),  fb = f*(b1 + b2),  f = sigmoid(w)

All heavy matmuls run in fp8e4 with MatmulPerfMode.DoubleRow (2 k-tiles
per instruction, 2x PE rate).  The 1/R row normalization is applied to
the *stationary* operand only (sc = E * C1/R), so the KQ Gram matmuls
for token-tile i depend only on tile i's exp + rowsum: exp, colsum and
KQ stream together per 2-tile pair with no softmax barrier.  KQ_raw is
symmetric: only the upper block-triangle is accumulated; lower blocks
are PE-transposed + rescaled (S[lo]/S[hi]) on the copy-out.

Scale plan (fp8 ranges):
    g   = E                    in [0.008, 250]    fp8
    sc  = E * C1/R, C1=1024    <= ~280            fp8
    kqt = (C1*KQraw) * (CKQ/C1)/S = CKQ*KQ, CKQ=256   ~0.3   fp8
    wts = 32*f*W^T             ~ +-0.9            fp8 (cast on host)
    msum = (CKQ*32*Msum)/8 = 1024*Msum            ~ +-2.7    fp8
    xti = 8*x^T  (scale riding the psum->sbuf copy)          fp8
    y_ps = xti @ msum + (ones/128)^T @ (8192*fb)  = 8192*(x@Msum + fb)
    out = y_ps/8192 + x   (exact f32 residual), written bf16

Distribution: pure data-parallel - batch b -> core b, no collectives.
DMA: x2/x3/x stream as 2-tile transfers alternating over both HWDGE
queues (sync + scalar); weights ride gpsimd gated behind the first
input pair (so they don't steal HBM bandwidth from the latency-critical
first exp); outputs ride gpsimd cast-DMAs (f32 SBUF -> bf16 HBM).
"""

import numpy as np
import ml_dtypes

import concourse.bass as bass
import concourse.tile as tile
from concourse import bacc
from concourse import mybir
from concourse.bass_utils import run_bass_kernel_spmd

F32 = mybir.dt.float32
F32R = mybir.dt.float32r
BF16 = mybir.dt.bfloat16
FP8 = mybir.dt.float8e4
DR = mybir.MatmulPerfMode.DoubleRow

B = 8
P = 128
D = 768
DT = D // P  # 6 feature tiles
C1 = 1024.0
CKQ = 256.0
CW = 32.0
CM_DIV = 8.0          # msum = msum_ps / 8
CY = 8.0              # xti = 8*x^T
Y_SCALE = 1.0 / (CKQ * CW / CM_DIV * CY)  # 1/8192

# moving-dim chunks: each must stay inside one PSUM bank (512 f32)
CHUNKS = ((0, 512), (512, 256))
# upper-triangle chunk plan per d'-tile: (col_off, width, psum_off), offsets
# packed so no chunk crosses a 512-f32 bank boundary
SYM_PLAN = {
    0: ((0, 512, 0), (512, 256, 512)),
    1: ((128, 384, 1024), (512, 256, 768)),
    2: ((256, 256, 1536), (512, 256, 1792)),
    3: ((256, 256, 2048), (512, 256, 2304)),
    4: ((512, 256, 2560),),
    5: ((512, 256, 2816),),
}
# contiguous (dst_col, src_psum_off, width) copy-out plan per dp
KQ_COPY = {
    0: ((0, 0, 768),),
    1: ((128, 1024, 384), (512, 768, 256)),
    2: ((256, 1536, 512),),
    3: ((256, 2048, 512),),
    4: ((512, 2560, 256),),
    5: ((512, 2816, 256),),
}
SYM_FILLS = [
    (1, 0), (2, 0), (2, 1), (3, 0), (3, 1),
    (4, 0), (4, 1), (4, 2), (4, 3),
    (5, 0), (5, 1), (5, 2), (5, 3),
]


def build_nc(NT=16):
    """Build the single-core Bass program.  NT = number of 128-token tiles."""
    N = NT * P
    NP = NT // 2  # tile pairs
    nc = bacc.Bacc()

    x_d = nc.dram_tensor("x", [N, D], F32, kind="ExternalInput")
    x2_d = nc.dram_tensor("x2", [N, D], F32, kind="ExternalInput")
    x3_d = nc.dram_tensor("x3", [N, D], F32, kind="ExternalInput")
    wt1_d = nc.dram_tensor("wt1", [D, D], FP8, kind="ExternalInput")  # 32*f*W1^T
    wt2_d = nc.dram_tensor("wt2", [D, D], FP8, kind="ExternalInput")  # 32*f*W2^T
    fb_d = nc.dram_tensor("fb", [1, D], F32, kind="ExternalInput")  # 8192*f*(b1+b2)
    id_d = nc.dram_tensor("ident", [P, P], F32, kind="ExternalInput")  # np.eye
    out_d = nc.dram_tensor("out", [N, D], BF16, kind="ExternalOutput")

    # pair-granular views: [pair, p, 2, d]
    x2_pr = x2_d.rearrange("(q t p) d -> q p t d", t=2, p=P)
    x3_pr = x3_d.rearrange("(q t p) d -> q p t d", t=2, p=P)
    x_pr = x_d.rearrange("(q t p) d -> q p t d", t=2, p=P)
    att_pr = [x2_pr, x3_pr]
    out_t = out_d.rearrange("(t p) d -> t p d", p=P)

    with tile.TileContext(nc) as tc:
        with (
            # one statically packed PSUM region (all 8 banks), sliced manually
            tc.tile_pool(name="ps", bufs=1, space="PSUM") as psp,
            tc.tile_pool(name="consts", bufs=1) as consts,
            tc.tile_pool(name="big", bufs=2) as big,
            tc.tile_pool(name="stream", bufs=4) as stream,
            tc.tile_pool(name="scp", bufs=4) as scp,
            tc.tile_pool(name="stats", bufs=2) as stats,
            tc.tile_pool(name="xtip", bufs=3) as xtip,
            tc.tile_pool(name="outp", bufs=3) as outp,
        ):
            # ---- PSUM layout (f32 offsets; banks are 512 f32) ----
            # phase A: [0..3072) KQ upper-tri chunks, [3072..3840) s_ps colsum,
            #          [3840..4096) two [P,128] transpose slots
            # phase B: m_ps double-buffered at 0 / 1024
            # phase C: y_ps at 0 / 1024, x^T batches at 2048 / 3072
            psb = psp.tile([P, 4096], F32)
            TP_OFF = (3840, 3968)

            # ---- constants ----
            ones8 = consts.tile([P, 2, P], FP8)
            nc.vector.memset(ones8, 1.0)
            onesfb = consts.tile([P, P], BF16)
            nc.vector.memset(onesfb, 1.0 / 128.0)
            ident = consts.tile([P, P], F32)
            identr = consts.tile([P, P], F32R)
            ident8 = consts.tile([P, P], FP8)
            fbb = consts.tile([P, D], F32)
            fbby = consts.tile([P, D], BF16)
            wts = consts.tile([P, 2, DT, D], FP8)
            kqt = [
                consts.tile([P, DT, D], FP8, tag=f"kqt{t}", name=f"kqt{t}")
                for t in range(2)
            ]
            msum = consts.tile([P, DT, D], FP8)
            rscol = consts.tile([P, 2, DT], F32)  # (CKQ/C1)/S per d'
            xbig = consts.tile([P, NT, D], F32)
            gate = consts.tile([P, 1], F32)

            nc.sync.dma_start(out=ident, in_=id_d[:, :])
            nc.vector.tensor_copy(identr, ident)
            nc.vector.tensor_copy(ident8, ident)
            nc.sync.dma_start(out=fbb, in_=fb_d[0:1, :].to_broadcast([P, D]))
            nc.vector.tensor_copy(fbby, fbb)

            # ---- per-attention phases: exp + colsum + KQ streamed ----
            for t in range(2):
                g = big.tile([P, NT, D], FP8, tag="big", name=f"g{t}")
                rvec = stats.tile([P, NT], F32, tag="rvec")
                rrec = stats.tile([P, NT], F32, tag="rrec")
                s_ps = psb[:, 3072:3840]
                kq_ps = psb[:, 0:3072]
                for q in range(NP):
                    xi = stream.tile([P, 2, D], F32, tag="in")
                    eng = nc.sync if q % 2 == 0 else nc.scalar
                    eng.dma_start(out=xi, in_=att_pr[t][q])
                    if t == 0 and q == 0:
                        # gate the gpsimd weight DMAs behind the first input
                        # pair so they don't compete for HBM with it
                        nc.gpsimd.dma_start(out=gate, in_=xi[:, 0, 0:1])
                        for tw, wd in enumerate((wt1_d, wt2_d)):
                            nc.gpsimd.dma_start(
                                out=wts[:, tw],
                                in_=wd.rearrange("(c p) j -> p c j", p=P),
                            )
                    sc = scp.tile([P, 2, D], FP8, tag="sc")
                    for j in range(2):
                        i = 2 * q + j
                        nc.scalar.activation(
                            out=g[:, i, :], in_=xi[:, j, :],
                            func=mybir.ActivationFunctionType.Exp,
                            accum_out=rvec[:, i : i + 1],
                        )
                    nc.vector.reciprocal(rrec[:, 2 * q : 2 * q + 2],
                                         rvec[:, 2 * q : 2 * q + 2])
                    for j in range(2):
                        i = 2 * q + j
                        nc.vector.tensor_scalar(
                            out=sc[:, j, :], in0=g[:, i, :],
                            scalar1=rrec[:, i : i + 1], scalar2=C1,
                            op0=mybir.AluOpType.mult, op1=mybir.AluOpType.mult,
                        )
                    # colsum S accumulates via ones-matmul (DoubleRow pair)
                    for off, sz in CHUNKS:
                        nc.tensor.matmul(
                            s_ps[:, off : off + sz],
                            ones8,
                            g[:, 2 * q : 2 * q + 2, off : off + sz],
                            start=(q == 0), stop=(q == NP - 1),
                            perf_mode=DR,
                        )
                    # KQ upper block-triangle, 2 token-tiles per matmul
                    for dp in range(DT):
                        lhsT = sc[:, :, dp * P : (dp + 1) * P]
                        for off, sz, poff in SYM_PLAN[dp]:
                            nc.tensor.matmul(
                                kq_ps[:, poff : poff + sz],
                                lhsT,
                                g[:, 2 * q : 2 * q + 2, off : off + sz],
                                start=(q == 0), stop=(q == NP - 1),
                                perf_mode=DR,
                            )

                # 1/S (scaled) + S*4 for the symmetric fills
                rsb = stream.tile([P, D], F32, tag="rsb", bufs=2)
                nc.vector.reciprocal(rsb, s_ps)
                nc.vector.tensor_scalar_mul(rsb, rsb, CKQ / C1)
                ssb = stream.tile([P, D], F32, tag="ssb", bufs=2)
                nc.vector.tensor_scalar_mul(ssb, s_ps, C1 / CKQ)
                for c in range(DT):
                    tp = psb[:, TP_OFF[c % 2] : TP_OFF[c % 2] + P]
                    nc.tensor.transpose(tp, rsb[:, c * P : (c + 1) * P], ident)
                    nc.vector.tensor_copy(rscol[:, t, c : c + 1], tp[:, 0:1])

                # upper blocks: scale rows by (CKQ/C1)/S[d'] on the psum copy
                for dp in range(DT):
                    for dst, src, w in KQ_COPY[dp]:
                        nc.vector.tensor_scalar_mul(
                            kqt[t][:, dp, dst : dst + w],
                            kq_ps[:, src : src + w],
                            rscol[:, t, dp : dp + 1],
                        )
                # lower blocks = transposed upper blocks rescaled:
                # kqt[hi][p, lo*P+q] = tp[p, q] * S[lo*P+q]*(C1/CKQ) * rscol[hi]
                for fi, (hi, lo) in enumerate(SYM_FILLS):
                    tp = psb[:, TP_OFF[fi % 2] : TP_OFF[fi % 2] + P]
                    # transpose via plain matmul (lhsT^T @ I) into f32 psum
                    nc.tensor.matmul(
                        tp, kqt[t][:, lo, hi * P : (hi + 1) * P], ident8,
                        start=True, stop=True,
                    )
                    nc.vector.scalar_tensor_tensor(
                        out=kqt[t][:, hi, lo * P : (lo + 1) * P],
                        in0=tp,
                        scalar=rscol[:, t, hi : hi + 1],
                        in1=ssb[:, lo * P : (lo + 1) * P],
                        op0=mybir.AluOpType.mult,
                        op1=mybir.AluOpType.mult,
                    )

                # x prefetch rides both HWDGE queues behind x3
                if t == 1:
                    for q in range(NP):
                        eng = nc.sync if q % 2 == 0 else nc.scalar
                        eng.dma_start(out=xbig[:, 2 * q : 2 * q + 2, :],
                                      in_=x_pr[q])

            # ---- Msum[d, j] = sum_t sum_d' kqt[t][d', d] * wts[t][d', j] ----
            for d in range(DT):
                mb = 1024 * (d % 2)
                m_ps = psb[:, mb : mb + D]
                for t in range(2):
                    for dpp in range(0, DT, 2):
                        lhsT = kqt[t][:, dpp : dpp + 2, d * P : (d + 1) * P]
                        for off, sz in CHUNKS:
                            nc.tensor.matmul(
                                m_ps[:, off : off + sz],
                                lhsT,
                                wts[:, t, dpp : dpp + 2, off : off + sz],
                                start=(t == 0 and dpp == 0),
                                stop=(t == 1 and dpp == DT - 2),
                                perf_mode=DR,
                            )
                eng = nc.vector if d % 2 == 0 else nc.scalar
                if d % 2 == 0:
                    nc.vector.tensor_scalar_mul(msum[:, d, :], m_ps, 1.0 / CM_DIV)
                else:
                    nc.scalar.mul(msum[:, d, :], m_ps, 1.0 / CM_DIV)

            # ---- y = xti @ msum + fb_mm; out = y/8192 + x ----
            for i in range(NT):
                # 6 transposes land in one contiguous psum region -> single
                # scaled copy (f32 -> 8*x^T in fp8)
                xb = 2048 + 1024 * (i % 2)
                xt_ps = psb[:, xb : xb + D]
                for c in range(DT):
                    nc.tensor.transpose(
                        xt_ps[:, c * P : (c + 1) * P],
                        xbig[:, i, c * P : (c + 1) * P],
                        ident,
                    )
                xti = xtip.tile([P, DT, P], FP8, tag="xti")
                if i % 2 == 0:
                    nc.scalar.mul(xti, xt_ps, CY)
                else:
                    nc.vector.tensor_scalar_mul(xti, xt_ps, CY)
                yb = 1024 * (i % 2)
                y_ps = psb[:, yb : yb + D]
                for off, sz in CHUNKS:
                    nc.tensor.matmul(
                        y_ps[:, off : off + sz], onesfb, fbby[:, off : off + sz],
                        start=True, stop=False,
                    )
                for k in range(0, DT, 2):
                    for off, sz in CHUNKS:
                        nc.tensor.matmul(
                            y_ps[:, off : off + sz],
                            xti[:, k : k + 2, :],
                            msum[:, k : k + 2, off : off + sz],
                            start=False, stop=(k == DT - 2),
                            perf_mode=DR,
                        )
                oi = outp.tile([P, D], F32, tag="out")
                nc.vector.scalar_tensor_tensor(
                    out=oi, in0=y_ps, scalar=Y_SCALE, in1=xbig[:, i, :],
                    op0=mybir.AluOpType.mult, op1=mybir.AluOpType.add,
                )
                # f32 -> bf16 cast rides the gpsimd store
                nc.gpsimd.dma_start(out=out_t[i], in_=oi)

    nc.compile()
    return nc


def prep_inputs(inputs):
    x = np.ascontiguousarray(np.asarray(inputs["x"], dtype=np.float32))
    x2 = np.ascontiguousarray(np.asarray(inputs["x2"], dtype=np.float32))
    x3 = np.ascontiguousarray(np.asarray(inputs["x3"], dtype=np.float32))
    W1 = np.asarray(inputs["W1"], dtype=np.float32)
    b1 = np.asarray(inputs["b1"], dtype=np.float32)
    W2 = np.asarray(inputs["W2"], dtype=np.float32)
    b2 = np.asarray(inputs["b2"], dtype=np.float32)
    w = np.asarray(inputs["w"], dtype=np.float32)

    f = 1.0 / (1.0 + np.exp(-float(w.reshape(-1)[0])))
    wt1 = np.ascontiguousarray((CW * f * W1).T).astype(ml_dtypes.float8_e4m3fn)
    wt2 = np.ascontiguousarray((CW * f * W2).T).astype(ml_dtypes.float8_e4m3fn)
    fb = (f * (b1 + b2) / Y_SCALE).astype(np.float32).reshape(1, D)

    ident = np.eye(P, dtype=np.float32)
    return [
        {
            "x": x[b], "x2": x2[b], "x3": x3[b],
            "wt1": wt1, "wt2": wt2, "fb": fb, "ident": ident,
        }
        for b in range(B)
    ]


_NC = None


def kernel(**inputs) -> np.ndarray:
    global _NC
    if _NC is None:
        _NC = build_nc()
    in_maps = prep_inputs(inputs)
    res = run_bass_kernel_spmd(_NC, in_maps, list(range(B)))
    return np.stack(
        [res.results[b]["out"].astype(np.float32) for b in range(B)], axis=0
    )
